# revision 1
# baseline (speedup 1.0000x reference)
"""GCN (3-layer + mean-pool + linear head) on 8 TRN2 NeuronCores.

Strategy (dst-sharded message passing):
  - Nodes are split into 8 contiguous slices of 12500; core i owns dst slice i
    and processes the edges that point into it.
  - norm = dis[src]*dis[dst] with dis = deg^-1/2 is folded into per-node row
    scalings (dis applied to GEMM output before gather, and to the aggregation
    output), so edge messages need no per-edge scaling.
  - Aggregation out[dst] += m[src]: DMA-gather of 512-byte message rows
    (f32 split exactly into bf16 hi+lo planes) into edge-tile layout, then
    one-hot fp8 "segment matrix" matmuls on the TensorEngine accumulate 128
    edges per instruction into PSUM per 128-dst block.
  - Layer 1 aggregates x first (3 features), then applies W1: saves gather
    bytes ((S x) W1 == S (x W1)).
  - Layer 3 + mean-pool collapse into a dense matmul: pooled = C' @ (h2 W3)
    with C'[g,j] = dis[j] * sum_{j->i, batch[i]=g} dis[i] / cnt_g, which is
    pure index/degree preprocessing. This removes layer 3's gather entirely.
  - One AllGather (m2 message plane) + one AllReduce ([128,64] pooled partial).
  - int16 gather indices => sources processed in 4 chunks of 25000 rows.
  - All 8 cores run one SPMD program; per-(block,chunk) edge-tile counts are
    equalized across cores (padding with idx=0 / zero seg rows).
"""
import os
import sys

sys.path.insert(0, "/opt/trn_rl_repo")

import numpy as np
import ml_dtypes

N = 100000
E = 1600000
F = 3
H = 128
C = 4
G = 64
P = 8
NP = N // P            # 12500 nodes per core
BLK = 128
NBLK = (NP + BLK - 1) // BLK   # 98 (last block has 84 rows)
BPG = 4                        # dst blocks per group
NGRP = (NBLK + BPG - 1) // BPG # 25
NCHUNK = 4
CHUNK = N // NCHUNK            # 25000
NPADN = 100096                 # 782*128 (x prep padding)
XCOLS = NPADN * F // 128       # 2346
ELEM1 = 128                    # bf16 elems per x_hl row (256B)
ELEM2 = 256                    # bf16 elems per m_hl row (512B)

_CACHE = {}


def _host_prep(x, edge_index, batch):
    f8 = ml_dtypes.float8_e4m3
    src = np.concatenate([edge_index[0], np.arange(N, dtype=np.int64)])
    dst = np.concatenate([edge_index[1], np.arange(N, dtype=np.int64)])
    deg = np.bincount(dst, minlength=N).astype(np.float32)
    dis = np.where(deg > 0, deg ** np.float32(-0.5), np.float32(0)).astype(np.float32)

    core = dst // NP
    blk = (dst % NP) // BLK
    dloc = (dst % NP) % BLK
    chunk = src // CHUNK
    srel = (src % CHUNK).astype(np.int16)

    # per (core, blk, chunk) counts -> equalized tile counts
    key = (core * NBLK + blk) * NCHUNK + chunk
    cnt = np.bincount(key, minlength=P * NBLK * NCHUNK).reshape(P, NBLK, NCHUNK)
    Tbc = np.maximum(1, -(-cnt.max(axis=0) // BLK)).astype(np.int64)  # [NBLK, NCHUNK]

    # tile layout: iterate g, c, blocks-in-group; record offsets
    groups = [list(range(g * BPG, min((g + 1) * BPG, NBLK))) for g in range(NGRP)]
    tile_off = np.zeros((NBLK, NCHUNK), np.int64)
    gc_base = np.zeros((NGRP, NCHUNK), np.int64)
    gc_ntiles = np.zeros((NGRP, NCHUNK), np.int64)
    tt = 0
    for g in range(NGRP):
        for c in range(NCHUNK):
            gc_base[g, c] = tt
            for b in groups[g]:
                tile_off[b, c] = tt
                tt += Tbc[b, c]
            gc_ntiles[g, c] = tt - gc_base[g, c]
    TOT = tt
    meta = dict(groups=groups, Tbc=Tbc, tile_off=tile_off, gc_base=gc_base,
                gc_ntiles=gc_ntiles, TOT=TOT,
                TGC_MAX=int(gc_ntiles.max()))

    # shared small tensors
    xp = np.zeros(NPADN * F, np.float32)
    xp[: N * F] = np.asarray(x, np.float32).reshape(-1)
    x_pad = xp.reshape(128, XCOLS)
    disp = np.zeros(NPADN, np.float32)
    disp[:N] = dis
    dis3 = np.repeat(disp, F).reshape(128, XCOLS)

    # C' pooled matrix: C'[g_, j] = dis[j]*sum_{edges j->i, batch[i]=g_} dis[i] / cnt_g
    batch = np.asarray(batch, np.int64)
    cntg = np.bincount(batch, minlength=G).astype(np.float32)
    cmat = np.zeros((G, N), np.float32)
    np.add.at(cmat, (batch[dst], src), (dis[src] * dis[dst]).astype(np.float32))
    cmat /= np.maximum(cntg, 1.0)[:, None]

    # per-core arrays
    per_core = []
    order_all = np.argsort(core, kind="stable")
    bounds = np.searchsorted(core[order_all], np.arange(P + 1))
    for i in range(P):
        sel = order_all[bounds[i]:bounds[i + 1]]
        eb, ec, ed, es = blk[sel], chunk[sel], dloc[sel], srel[sel]
        bucket = eb * NCHUNK + ec
        o2 = np.argsort(bucket, kind="stable")
        eb, ec, ed, es, bucket = eb[o2], ec[o2], ed[o2], es[o2], bucket[o2]
        # rank within bucket
        bc = np.bincount(bucket, minlength=NBLK * NCHUNK)
        starts = np.zeros(NBLK * NCHUNK, np.int64)
        starts[1:] = np.cumsum(bc)[:-1]
        rank = np.arange(len(sel)) - starts[bucket]
        slot = tile_off.reshape(-1)[bucket] * BLK + rank
        # idx stream
        idx16 = np.zeros(TOT * BLK, np.int16)
        idx16[slot] = es
        idxw = np.zeros((128, TOT * 8), np.int16)
        jj = np.arange(TOT * BLK)
        base = idx16[jj].reshape(-1, 16)
        colbase = (jj // 16).reshape(-1, 16)[:, 0]
        for r in range(8):
            idxw[16 * r:16 * r + 16, :] = 0
        wr = idx16.reshape(-1, 16).T  # [16, TOT*8]
        for r in range(8):
            idxw[16 * r:16 * r + 16, :] = wr
        # seg one-hot
        seg = np.zeros((128, TOT * BLK), f8)
        seg[slot % BLK, (slot // BLK) * BLK + ed] = np.float32(1.0).astype(f8)
        # dis_own [128, NBLK]
        dso = np.zeros((128, NBLK), np.float32)
        own = dis[i * NP:(i + 1) * NP]
        dso.T.reshape(-1)[: NP] = own
        dso = np.zeros(NBLK * 128, np.float32)
        dso[:NP] = own
        dso = dso.reshape(NBLK, 128).T.copy()  # [128, NBLK]
        # cp [128, NBLK*64]
        cpo = np.zeros(NBLK * 128, np.float32)
        cpc = np.zeros((NBLK * 128, G), np.float32)
        cpc[:NP, :] = cmat[:, i * NP:(i + 1) * NP].T
        cp = cpc.reshape(NBLK, 128, G).transpose(1, 0, 2).reshape(128, NBLK * G).copy()
        per_core.append(dict(idxw=idxw, seg=np.asarray(seg), dso=dso, cp=cp))

    return meta, dict(x_pad=x_pad, dis3=dis3), per_core


def _build(meta, W_shapes):
    import concourse.bacc as bacc
    import concourse.mybir as mybir
    import concourse.tile as tile

    dt = mybir.dt
    AF = mybir.ActivationFunctionType
    ALU = mybir.AluOpType

    groups = meta["groups"]
    Tbc = meta["Tbc"]
    gc_base = meta["gc_base"]
    gc_ntiles = meta["gc_ntiles"]
    TOT = meta["TOT"]
    TGC_MAX = meta["TGC_MAX"]

    nc = bacc.Bacc("TRN2", target_bir_lowering=False, debug=False,
                   num_devices=P, num_swdge_queues=4)

    # ---- dram tensors ----
    t_xpad = nc.dram_tensor("x_pad", [128, XCOLS], dt.float32, kind="ExternalInput").ap()
    t_dis3 = nc.dram_tensor("dis3", [128, XCOLS], dt.float32, kind="ExternalInput").ap()
    t_idx = nc.dram_tensor("idxw", [128, TOT * 8], dt.int16, kind="ExternalInput").ap()
    t_seg = nc.dram_tensor("seg", [128, TOT * BLK], dt.float8e4, kind="ExternalInput").ap()
    t_dso = nc.dram_tensor("dso", [128, NBLK], dt.float32, kind="ExternalInput").ap()
    t_cp = nc.dram_tensor("cp", [128, NBLK * G], dt.float32, kind="ExternalInput").ap()
    t_w1 = nc.dram_tensor("w1", [2 * F, H], dt.float32, kind="ExternalInput").ap()
    t_w2 = nc.dram_tensor("w2", [H, H], dt.float32, kind="ExternalInput").ap()
    t_w3 = nc.dram_tensor("w3", [H, H], dt.float32, kind="ExternalInput").ap()
    t_wl = nc.dram_tensor("wl", [H, C], dt.float32, kind="ExternalInput").ap()
    t_b1 = nc.dram_tensor("b1b", [128, H], dt.float32, kind="ExternalInput").ap()
    t_b2 = nc.dram_tensor("b2b", [128, H], dt.float32, kind="ExternalInput").ap()
    t_b3 = nc.dram_tensor("b3c", [128, 1], dt.float32, kind="ExternalInput").ap()
    t_bl = nc.dram_tensor("blc", [C, 1], dt.float32, kind="ExternalInput").ap()
    t_id = nc.dram_tensor("ident", [128, 128], dt.float32, kind="ExternalInput").ap()
    t_out = nc.dram_tensor("out", [C, G], dt.float32, kind="ExternalOutput").ap()

    t_xhl = nc.dram_tensor("x_hl", [NPADN, ELEM1], dt.bfloat16, kind="Internal").ap()
    t_min = nc.dram_tensor("mhl_in", [NP, ELEM2], dt.bfloat16, kind="Internal").ap()
    t_mfull = nc.dram_tensor("mhl_full", [N, ELEM2], dt.bfloat16, kind="Internal",
                             addr_space="Shared").ap()
    t_arin = nc.dram_tensor("arin", [128, G], dt.float32, kind="Internal").ap()
    t_arout = nc.dram_tensor("arout", [128, G], dt.float32, kind="Internal",
                             addr_space="Shared").ap()
    debug = os.environ.get("GCN_DEBUG", "0") == "1"
    t_dh1 = t_dh2 = None
    if debug:
        t_dh1 = nc.dram_tensor("dbg_h1", [NP, H], dt.float32, kind="ExternalOutput").ap()
        t_dh2 = nc.dram_tensor("dbg_h2", [NP, H], dt.float32, kind="ExternalOutput").ap()

    qctr = [0]

    def nextq():
        q = qctr[0] % 4
        qctr[0] += 1
        return q

    with tile.TileContext(nc) as tc:
        with tc.tile_pool(name="const", bufs=1) as cpool:
            w1s = cpool.tile([2 * F, H], dt.float32)
            w2s = cpool.tile([H, H], dt.float32)
            w3s = cpool.tile([H, H], dt.float32)
            wls = cpool.tile([H, C], dt.float32)
            b1s = cpool.tile([128, H], dt.float32)
            b2s = cpool.tile([128, H], dt.float32)
            b3s = cpool.tile([128, 1], dt.float32)
            bls = cpool.tile([C, 1], dt.float32)
            dsos = cpool.tile([128, NBLK], dt.float32)
            cps = cpool.tile([128, NBLK * G], dt.float32)
            ids = cpool.tile([128, 128], dt.float32)
            for dst_t, src_t in [(w1s, t_w1), (w2s, t_w2), (w3s, t_w3), (wls, t_wl),
                                 (b1s, t_b1), (b2s, t_b2), (b3s, t_b3), (bls, t_bl),
                                 (dsos, t_dso), (cps, t_cp), (ids, t_id)]:
                nc.sync.dma_start(dst_t[:], src_t[:])

            # ---- phase X: build x_hl = [bf16(dis*x) | bf16(residual)] rows ----
            with tc.tile_pool(name="xprep", bufs=1) as xp:
                xf = xp.tile([128, XCOLS], dt.float32)
                d3 = xp.tile([128, XCOLS], dt.float32)
                xm = xp.tile([128, XCOLS], dt.float32)
                h32 = xp.tile([128, XCOLS], dt.float32)
                hilo = xp.tile([128, 2 * XCOLS], dt.bfloat16)
                nc.sync.dma_start(xf[:], t_xpad[:])
                nc.sync.dma_start(d3[:], t_dis3[:])
                nc.vector.tensor_tensor(xm[:], xf[:], d3[:], op=ALU.mult)
                hiv = hilo[:].rearrange("p (t c) -> p t c", c=2 * F)[:, :, 0:F]
                lov = hilo[:].rearrange("p (t c) -> p t c", c=2 * F)[:, :, F:2 * F]
                xmv = xm[:].rearrange("p (t c) -> p t c", c=F)
                nc.vector.tensor_copy(hiv, xmv)
                nc.vector.tensor_copy(h32[:].rearrange("p (t c) -> p t c", c=F), hiv)
                nc.vector.tensor_tensor(lov, xmv,
                                        h32[:].rearrange("p (t c) -> p t c", c=F),
                                        op=ALU.subtract)
                xhv = t_xhl[:, 0:2 * F].rearrange("(p t) c -> p t c", p=128)
                hlv = hilo[:].rearrange("p (t c) -> p t c", c=2 * F)
                half = NPADN // 128 // 2  # 391
                nc.sync.dma_start(xhv[:, 0:half, :], hlv[:, 0:half, :])
                nc.sync.dma_start(xhv[:, half:2 * half, :], hlv[:, half:2 * half, :])

            # ---- phase L1 + L2-GEMM (per group / per block) ----
            with tc.tile_pool(name="g1", bufs=10) as g1p, \
                 tc.tile_pool(name="seg1", bufs=10) as seg1p, \
                 tc.tile_pool(name="idx1", bufs=10) as idx1p, \
                 tc.tile_pool(name="blk1", bufs=4) as blkp, \
                 tc.tile_pool(name="ps_z", bufs=2, space="PSUM") as psz, \
                 tc.tile_pool(name="ps_a", bufs=2, space="PSUM") as psa, \
                 tc.tile_pool(name="ps_a2", bufs=2, space="PSUM") as psa2, \
                 tc.tile_pool(name="ps_b", bufs=2, space="PSUM") as psb:
                for g in range(NGRP):
                    blocks = groups[g]
                    gtiles = []
                    segtiles = []
                    for c in range(NCHUNK):
                        ntile = int(gc_ntiles[g, c])
                        nidx = ntile * BLK
                        base = int(gc_base[g, c])
                        it = idx1p.tile([128, TGC_MAX * 8], dt.int16, tag="idx",
                                        name=f"i1_{g}_{c}")
                        st = seg1p.tile([128, TGC_MAX * BLK], dt.float8e4, tag="seg",
                                        name=f"s1_{g}_{c}")
                        gt = g1p.tile([128, TGC_MAX, ELEM1], dt.bfloat16, tag="g",
                                      name=f"g1_{g}_{c}")
                        nc.sync.dma_start(it[:, 0:nidx // 16],
                                          t_idx[:, base * 8:base * 8 + nidx // 16])
                        nc.sync.dma_start(st[:, 0:nidx],
                                          t_seg[:, base * BLK:base * BLK + nidx])
                        nc.gpsimd.dma_gather(
                            gt[:, 0:ntile, :], t_xhl[c * CHUNK:(c + 1) * CHUNK, :],
                            it[:, 0:nidx // 16], nidx, nidx, ELEM1,
                            single_packet=False, queue_num=nextq())
                        gtiles.append(gt)
                        segtiles.append(st)
                    zbanks = [psz.tile([2 * F, 512], dt.float32, tag="z", name=f"z{g}_{k}")
                              for k in range((len(blocks) + 3) // 4)]
                    for kb, b in enumerate(blocks):
                        reg = zbanks[kb // 4][:, (kb % 4) * 128:(kb % 4 + 1) * 128]
                        nmm = sum(int(Tbc[b, c]) for c in range(NCHUNK))
                        k = 0
                        for c in range(NCHUNK):
                            t0 = int(tile_off := (meta["tile_off"][b, c] - gc_base[g, c]))
                            for t in range(int(Tbc[b, c])):
                                nc.tensor.matmul(
                                    reg,
                                    lhsT=gtiles[c][:, t0 + t, 0:2 * F],
                                    rhs=segtiles[c][:, (t0 + t) * BLK:(t0 + t + 1) * BLK],
                                    start=(k == 0), stop=(k == nmm - 1))
                                k += 1
                        # z6 = [zhi; zlo]; hi+lo fold happens in the GEMM via [W1;W1]
                        zs = blkp.tile([2 * F, 128], dt.float32, tag="zs", name=f"zs{g}_{kb}")
                        nc.scalar.copy(zs[:], reg[0:2 * F, :])
                        # h1 = relu(dis * (z6^T @ [W1;W1]) + b1)
                        ph = psa.tile([128, H], dt.float32, tag="ph", name=f"ph{g}_{kb}")
                        nc.tensor.matmul(ph[:], lhsT=zs[:], rhs=w1s[:], start=True, stop=True)
                        h1 = blkp.tile([128, H], dt.float32, tag="h1", name=f"h1_{g}_{kb}")
                        nc.scalar.activation(h1[:], ph[:], AF.Copy,
                                             scale=dsos[:, b:b + 1])
                        nc.vector.tensor_tensor(h1[:], h1[:], b1s[:], op=ALU.add)
                        nc.scalar.activation(h1[:], h1[:], AF.Relu)
                        # transpose h1 -> h1fm
                        pt = psb.tile([128, 128], dt.float32, tag="pt", name=f"pt{g}_{kb}")
                        nc.tensor.transpose(pt[:], h1[:], ids[:])
                        h1f = blkp.tile([128, 128], dt.float32, tag="h1f", name=f"h1f{g}_{kb}")
                        nc.vector.tensor_copy(h1f[:], pt[:])
                        # m2 = dis * (h1 @ W2); split hi/lo bf16
                        pm = psa2.tile([128, H], dt.float32, tag="pm", name=f"pm{g}_{kb}")
                        nc.tensor.matmul(pm[:], lhsT=h1f[:], rhs=w2s[:], start=True, stop=True)
                        m2 = blkp.tile([128, H], dt.float32, tag="m2", name=f"m2_{g}_{kb}")
                        nc.scalar.activation(m2[:], pm[:], AF.Copy, scale=dsos[:, b:b + 1])
                        mhl = blkp.tile([128, ELEM2], dt.bfloat16, tag="mhl", name=f"mh{g}_{kb}")
                        m2h32 = blkp.tile([128, H], dt.float32, tag="m2h", name=f"m2h{g}_{kb}")
                        nc.vector.tensor_copy(mhl[:, 0:H], m2[:])
                        nc.vector.tensor_copy(m2h32[:], mhl[:, 0:H])
                        nc.vector.tensor_tensor(mhl[:, H:2 * H], m2[:], m2h32[:],
                                                op=ALU.subtract)
                        rb = min(128, NP - b * 128)
                        nc.sync.dma_start(t_min[b * 128:b * 128 + rb, :], mhl[0:rb, :])
                        if debug:
                            nc.sync.dma_start(t_dh1[b * 128:b * 128 + rb, :], h1[0:rb, :])

            # ---- AllGather m2 ----
            nc.gpsimd.collective_compute(
                "AllGather", mybir.AluOpType.bypass,
                replica_groups=[list(range(P))],
                ins=[t_min[:]], outs=[t_mfull[:]])

            # ---- phase L2 aggregation + L3 fold ----
            with tc.tile_pool(name="g2", bufs=8) as g2p, \
                 tc.tile_pool(name="seg2", bufs=8) as seg2p, \
                 tc.tile_pool(name="idx2", bufs=8) as idx2p, \
                 tc.tile_pool(name="blk2", bufs=4) as blk2p, \
                 tc.tile_pool(name="pool_acc", bufs=1) as pap, \
                 tc.tile_pool(name="ps_agg", bufs=4, space="PSUM") as psagg, \
                 tc.tile_pool(name="ps_c", bufs=2, space="PSUM") as psc, \
                 tc.tile_pool(name="ps_d", bufs=1, space="PSUM") as psd, \
                 tc.tile_pool(name="ps_e", bufs=1, space="PSUM") as pse:
                pacc = pap.tile([128, G], dt.float32)
                for g in range(NGRP):
                    blocks = groups[g]
                    gtiles = []
                    segtiles = []
                    for c in range(NCHUNK):
                        ntile = int(gc_ntiles[g, c])
                        nidx = ntile * BLK
                        base = int(gc_base[g, c])
                        it = idx2p.tile([128, TGC_MAX * 8], dt.int16, tag="idx",
                                        name=f"i2_{g}_{c}")
                        st = seg2p.tile([128, TGC_MAX * BLK], dt.float8e4, tag="seg",
                                        name=f"s2_{g}_{c}")
                        gt = g2p.tile([128, TGC_MAX, ELEM2], dt.bfloat16, tag="g",
                                      name=f"g2_{g}_{c}")
                        nc.sync.dma_start(it[:, 0:nidx // 16],
                                          t_idx[:, base * 8:base * 8 + nidx // 16])
                        nc.sync.dma_start(st[:, 0:nidx],
                                          t_seg[:, base * BLK:base * BLK + nidx])
                        nc.gpsimd.dma_gather(
                            gt[:, 0:ntile, :], t_mfull[c * CHUNK:(c + 1) * CHUNK, :],
                            it[:, 0:nidx // 16], nidx, nidx, ELEM2,
                            single_packet=False, queue_num=nextq())
                        gtiles.append(gt)
                        segtiles.append(st)
                    banks = [psagg.tile([128, 512], dt.float32, tag="agg",
                                        name=f"ab{g}_{k}")
                             for k in range((len(blocks) + 1) // 2)]
                    for kb, b in enumerate(blocks):
                        reg = banks[kb // 2][:, (kb % 2) * 256:(kb % 2 + 1) * 256]
                        nmm = sum(int(Tbc[b, c]) for c in range(NCHUNK))
                        k = 0
                        for c in range(NCHUNK):
                            t0 = int(meta["tile_off"][b, c] - gc_base[g, c])
                            for t in range(int(Tbc[b, c])):
                                nc.tensor.matmul(
                                    reg,
                                    lhsT=segtiles[c][:, (t0 + t) * BLK:(t0 + t + 1) * BLK],
                                    rhs=gtiles[c][:, t0 + t, :],
                                    start=(k == 0), stop=(k == nmm - 1))
                                k += 1
                        h2 = blk2p.tile([128, H], dt.float32, tag="h2", name=f"h2_{g}_{kb}")
                        nc.scalar.copy(h2[:], reg[:, 0:H])
                        nc.vector.tensor_tensor(h2[:], reg[:, H:2 * H], h2[:], op=ALU.add)
                        # h2 = relu(dis*h2 + b2)
                        nc.scalar.activation(h2[:], h2[:], AF.Copy, scale=dsos[:, b:b + 1])
                        nc.vector.tensor_tensor(h2[:], h2[:], b2s[:], op=ALU.add)
                        nc.scalar.activation(h2[:], h2[:], AF.Relu)
                        if debug:
                            rb = min(128, NP - b * 128)
                            nc.sync.dma_start(t_dh2[b * 128:b * 128 + rb, :], h2[0:rb, :])
                        pt = psc.tile([128, 128], dt.float32, tag="pt2", name=f"pt2{g}_{kb}")
                        nc.tensor.transpose(pt[:], h2[:], ids[:])
                        h2f = blk2p.tile([128, 128], dt.float32, tag="h2f", name=f"h2f{g}_{kb}")
                        nc.vector.tensor_copy(h2f[:], pt[:])
                        pm = psd.tile([128, H], dt.float32, tag="pm3", name=f"pm3{g}_{kb}")
                        nc.tensor.matmul(pm[:], lhsT=h2f[:], rhs=w3s[:], start=True, stop=True)
                        m3 = blk2p.tile([128, H], dt.float32, tag="m3", name=f"m3_{g}_{kb}")
                        nc.scalar.copy(m3[:], pm[:])
                        pp = pse.tile([128, G], dt.float32, tag="pp", name=f"pp{g}_{kb}")
                        nc.tensor.matmul(pp[:], lhsT=m3[:], rhs=cps[:, b * G:(b + 1) * G],
                                         start=True, stop=True)
                        if g == 0 and kb == 0:
                            nc.scalar.copy(pacc[:], pp[:])
                        else:
                            nc.vector.tensor_tensor(pacc[:], pp[:], pacc[:], op=ALU.add)

                # ---- tail: AllReduce pooled partials ----
                nc.sync.dma_start(t_arin[:], pacc[:])
                nc.gpsimd.collective_compute(
                    "AllReduce", mybir.AluOpType.add,
                    replica_groups=[list(range(P))],
                    ins=[t_arin[:]], outs=[t_arout[:]])

            # ---- head (after L2 pools release) ----
            with tc.tile_pool(name="tail", bufs=1) as tp, \
                 tc.tile_pool(name="ps_o", bufs=1, space="PSUM") as pso:
                psb_ = tp.tile([128, G], dt.float32)
                nc.sync.dma_start(psb_[:], t_arout[:])
                psb2 = tp.tile([128, G], dt.float32)
                nc.scalar.activation(psb2[:], psb_[:], AF.Identity,
                                     bias=b3s[:, 0:1])
                po = pso.tile([C, G], dt.float32)
                nc.tensor.matmul(po[:], lhsT=wls[:], rhs=psb2[:], start=True, stop=True)
                osb = tp.tile([C, G], dt.float32)
                nc.scalar.activation(osb[:], po[:], AF.Identity, bias=bls[:, 0:1])
                nc.sync.dma_start(t_out[:], osb[:])

    nc.compile()
    return nc


def kernel(**inputs):
    from concourse.bass_utils import run_bass_kernel_spmd

    x = np.asarray(inputs["x"], np.float32)
    edge_index = np.asarray(inputs["edge_index"], np.int64)
    batch = np.asarray(inputs["batch"], np.int64)
    W1 = np.asarray(inputs["W1"], np.float32)
    b1 = np.asarray(inputs["b1"], np.float32)
    W2 = np.asarray(inputs["W2"], np.float32)
    b2 = np.asarray(inputs["b2"], np.float32)
    W3 = np.asarray(inputs["W3"], np.float32)
    b3 = np.asarray(inputs["b3"], np.float32)
    Wlin = np.asarray(inputs["Wlin"], np.float32)
    blin = np.asarray(inputs["blin"], np.float32)

    meta, shared, per_core = _host_prep(x, edge_index, batch)

    key = "nc"
    if key not in _CACHE:
        _CACHE[key] = _build(meta, None)
    nc = _CACHE[key]

    in_maps = []
    for i in range(P):
        pc = per_core[i]
        in_maps.append({
            "x_pad": shared["x_pad"], "dis3": shared["dis3"],
            "idxw": pc["idxw"], "seg": pc["seg"], "dso": pc["dso"], "cp": pc["cp"],
            "w1": np.vstack([W1, W1]).astype(np.float32), "w2": W2, "w3": W3, "wl": Wlin,
            "b1b": np.tile(b1, (128, 1)).astype(np.float32),
            "b2b": np.tile(b2, (128, 1)).astype(np.float32),
            "b3c": b3.reshape(128, 1).astype(np.float32),
            "blc": blin.reshape(C, 1).astype(np.float32),
            "ident": np.eye(128, dtype=np.float32),
        })

    trace = os.environ.get("GCN_TRACE", "0") == "1"
    res = run_bass_kernel_spmd(nc, in_maps, core_ids=list(range(P)), trace=trace)
    if trace:
        print("HW exec time:", res.exec_time_ns, "ns")
        if res.instructions_and_trace:
            print("trace:", res.instructions_and_trace[1])
    out = res.results[0]["out"]  # [4, 64]
    return np.ascontiguousarray(out.T).astype(np.float32)



# revision 3
# speedup vs baseline: 1.7534x; 1.7534x over previous
"""GCN (3-layer + mean-pool + linear head) on 8 TRN2 NeuronCores.

v2 strategy (dst-sharded message passing, packed gathers, pipelined collective):
  - Nodes split into 8 slices of 12500; core i owns dst slice i (98 blocks of
    128) and the edges pointing into it. Self-loops are NOT gathered: their
    contribution is added algebraically from data already on-chip.
  - Table row order R(src) = (core, quarter, offset) so the AllGather of m2
    can be split into 4 quarter collectives that fire during the L1 block
    loop; L2 runs chunk-major with SBUF bf16 accumulators so chunk-q work
    only depends on collective q.
  - L1 aggregates x-tilde = dis*x (hi/lo bf16 in 256B rows, host-precomputed
    table) first, then applies W1: (S x) W1 == S (x W1).
  - Messages m2 = dis*(h1 W2) stored as single bf16 (256B rows).
  - Gather streams are PACKED per (group, chunk): blocks share boundary tile
    columns; the one-hot seg pieces route edges to the right dst block, so
    descriptors ~= edges (no per-block ceil-to-128 padding).
  - Layer 3 + mean-pool collapse into a dense matmul with host-built C'
    (index/degree data only). Final head partials [C, G] are summed on host
    (removes the tail AllReduce); bias constants folded on host.
"""
import os
import sys

sys.path.insert(0, "/opt/trn_rl_repo")

import numpy as np
import ml_dtypes

N = 100000
E = 1600000
F = 3
H = 128
C = 4
G = 64
P = 8
NP = N // P                 # 12500
BLK = 128
NBLK = (NP + BLK - 1) // BLK    # 98
BPG = 4
NGRP = (NBLK + BPG - 1) // BPG  # 25
NQ = 4
QBLKS = [25, 25, 25, 23]        # blocks per quarter
QS = [3200, 3200, 3200, 2900]   # rows per quarter per core
QO = [0, 3200, 6400, 9600]
CH = [8 * s for s in QS]        # chunk table sizes
CB = [0, 25600, 51200, 76800]   # chunk table bases
QGRP = [6, 12, 18, 24]          # collective q fires after this group
ELEM = 128                      # bf16 elems per table row (256B)

_CACHE = {}


def _bf16_hilo(a):
    hi = a.astype(ml_dtypes.bfloat16)
    lo = (a - hi.astype(np.float32)).astype(ml_dtypes.bfloat16)
    return hi, lo


def _host_prep(x, edge_index, batch):
    f8 = ml_dtypes.float8_e4m3
    src = np.asarray(edge_index[0], np.int64)
    dst = np.asarray(edge_index[1], np.int64)
    deg = (np.bincount(dst, minlength=N) + 1).astype(np.float32)  # + self-loop
    dis = deg ** np.float32(-0.5)

    # table row mapping (core, quarter, offset)
    def rowmap(s):
        i = s // NP
        r = s % NP
        q = np.minimum(r // 3200, 3)
        qs = np.asarray(QS, np.int64)[q]
        qo = np.asarray(QO, np.int64)[q]
        cb = np.asarray(CB, np.int64)[q]
        srel = i * qs + (r - qo)
        return q, srel, cb + srel

    # x-tilde table [N, ELEM] bf16: cols 0:3 hi, 3:6 lo, rest zero
    xt = dis[:, None] * np.asarray(x, np.float32)
    _, _, trow = rowmap(np.arange(N, dtype=np.int64))
    xhl = np.zeros((N, ELEM), ml_dtypes.bfloat16)
    hi, lo = _bf16_hilo(xt)
    xhl[trow, 0:F] = hi
    xhl[trow, F:2 * F] = lo

    # C' pooled matrix (uses FULL edge set incl self-loops)
    batch = np.asarray(batch, np.int64)
    cntg = np.bincount(batch, minlength=G).astype(np.float32)
    srcL = np.concatenate([src, np.arange(N, dtype=np.int64)])
    dstL = np.concatenate([dst, np.arange(N, dtype=np.int64)])
    cmat = np.zeros((G, N), np.float32)
    np.add.at(cmat, (batch[dstL], srcL), dis[srcL] * dis[dstL])
    cmat /= np.maximum(cntg, 1.0)[:, None]

    # per-edge core/block/chunk decomposition (NO self-loops)
    core = dst // NP
    blk = (dst % NP) // BLK
    dloc = (dst % NP) % BLK
    ec, es, _ = rowmap(src)

    gi = blk // BPG
    bi = blk % BPG
    key = ((core * NGRP + gi) * NQ + ec) * BPG + bi
    order = np.argsort(key, kind="stable")

    # per-core per-(g,c,b) counts
    kk = ((core * NBLK + blk) * NQ + ec)
    cnt = np.bincount(kk, minlength=P * NBLK * NQ).reshape(P, NBLK, NQ)

    # per (g,c): packed stream; per-core block starts
    start = np.zeros((P, NBLK, NQ), np.int64)
    for g in range(NGRP):
        bs = range(g * BPG, min((g + 1) * BPG, NBLK))
        run = np.zeros((P, NQ), np.int64)
        for b in bs:
            start[:, b, :] = run
            run += cnt[:, b, :]
    cnt_pad = np.zeros((P, NGRP * BPG, NQ), np.int64)
    cnt_pad[:, :NBLK, :] = cnt
    cnt_gc = cnt_pad.reshape(P, NGRP, BPG, NQ).sum(axis=2)   # [P, g, c]
    NIDX = (-(-cnt_gc.max(axis=0) // BLK) * BLK).astype(np.int64)  # [g, c]
    NIDX = np.maximum(NIDX, BLK)
    NTILE = NIDX // BLK

    # piece (block-column) program, shared across cores
    T0 = (start.min(axis=0) // BLK).astype(np.int64)              # [NBLK, NQ]
    T1 = (-(-(start + cnt).max(axis=0) // BLK)).astype(np.int64)  # [NBLK, NQ]
    T1 = np.maximum(T1, T0 + 1)
    SPAN = T1 - T0

    IBASE = np.zeros((NGRP, NQ), np.int64)   # idx stream base (in idx units)
    PBASE = np.zeros((NGRP, NQ), np.int64)   # seg piece base
    POFF = np.zeros((NBLK, NQ), np.int64)    # piece offset within (g,c)
    it_, pt_ = 0, 0
    for g in range(NGRP):
        for c in range(NQ):
            IBASE[g, c] = it_
            PBASE[g, c] = pt_
            po = 0
            for b in range(g * BPG, min((g + 1) * BPG, NBLK)):
                POFF[b, c] = po
                po += int(SPAN[b, c])
            it_ += int(NIDX[g, c])
            pt_ += po
    ITOT, PTOT = it_, pt_

    meta = dict(NIDX=NIDX, NTILE=NTILE, T0=T0, T1=T1, IBASE=IBASE,
                PBASE=PBASE, POFF=POFF, ITOT=ITOT, PTOT=PTOT,
                NT_MAX=int(NTILE.max()))

    # per-core streams
    per_core = []
    cbound = np.searchsorted(core[order], np.arange(P + 1))
    for i in range(P):
        sel = order[cbound[i]:cbound[i + 1]]
        eb, ecc, edd, ess = blk[sel], ec[sel], dloc[sel], es[sel]
        bckt = (eb * NQ + ecc)
        # rank within (b, c) bucket: edges already sorted by (g,c,b) so each
        # (b,c) bucket is contiguous; rank = position - bucket start
        bc_cnt = np.bincount(bckt, minlength=NBLK * NQ)
        bc_start = np.zeros(NBLK * NQ, np.int64)
        ord2 = np.argsort(bckt, kind="stable")
        inv = np.empty_like(ord2)
        inv[ord2] = np.arange(len(sel))
        bc_start[1:] = np.cumsum(bc_cnt)[:-1]
        rank_sorted = np.arange(len(sel)) - bc_start[bckt[ord2]]
        rank = np.empty(len(sel), np.int64)
        rank[ord2] = rank_sorted
        # position within the packed (g,c) stream
        pos = start[i][eb, ecc] + rank
        gidx = eb // BPG
        spos = IBASE[gidx, ecc] + pos           # global idx-stream position
        idx16 = np.zeros(ITOT, np.int16)
        idx16[spos] = ess.astype(np.int16)
        # wrap for gpsimd: [128, ITOT//16], replicated 8x over partition rows
        wr = idx16.reshape(-1, 16).T            # [16, ITOT//16]
        idxw = np.tile(wr, (8, 1)).astype(np.int16)
        # seg pieces
        tc = pos // BLK
        prow = pos % BLK
        pidx = PBASE[gidx, ecc] + POFF[eb, ecc] + (tc - T0[eb, ecc])
        seg = np.zeros((BLK, PTOT * BLK), f8)
        seg[prow, pidx * BLK + edd] = np.float32(1.0).astype(f8)
        # dis per block layout [128, NBLK]
        own = dis[i * NP:(i + 1) * NP]
        dso = np.zeros(NBLK * BLK, np.float32)
        dso[:NP] = own
        dso = dso.reshape(NBLK, BLK).T.copy()
        # cp [128, NBLK*G] bf16
        cpc = np.zeros((NBLK * BLK, G), np.float32)
        cpc[:NP, :] = cmat[:, i * NP:(i + 1) * NP].T
        cp = cpc.reshape(NBLK, BLK, G).transpose(1, 0, 2).reshape(BLK, NBLK * G)
        cp = cp.astype(ml_dtypes.bfloat16)
        # x-tilde^T hi/lo for the L1 self term [6, NBLK*128] f32
        xti = np.zeros((2 * F, NBLK * BLK), np.float32)
        hi_i, lo_i = _bf16_hilo(xt[i * NP:(i + 1) * NP])
        xti[0:F, :NP] = hi_i.astype(np.float32).T
        xti[F:2 * F, :NP] = lo_i.astype(np.float32).T
        per_core.append(dict(idxw=idxw, seg=np.asarray(seg), dso=dso, cp=cp,
                             xti=xti))

    return meta, xhl, per_core, dis


def _build(meta):
    import concourse.bacc as bacc
    import concourse.mybir as mybir
    import concourse.tile as tile

    dt = mybir.dt
    AF = mybir.ActivationFunctionType
    ALU = mybir.AluOpType

    NIDX = meta["NIDX"]
    NTILE = meta["NTILE"]
    T0, T1 = meta["T0"], meta["T1"]
    IBASE, PBASE, POFF = meta["IBASE"], meta["PBASE"], meta["POFF"]
    ITOT, PTOT = meta["ITOT"], meta["PTOT"]
    NT_MAX = meta["NT_MAX"]

    nc = bacc.Bacc("TRN2", target_bir_lowering=False, debug=False,
                   num_devices=P, num_swdge_queues=4)

    t_xhl = nc.dram_tensor("xhl", [N, ELEM], dt.bfloat16, kind="ExternalInput").ap()
    t_idx = nc.dram_tensor("idxw", [128, ITOT // 16], dt.int16, kind="ExternalInput").ap()
    t_seg = nc.dram_tensor("seg", [128, PTOT * BLK], dt.float8e4, kind="ExternalInput").ap()
    t_dso = nc.dram_tensor("dso", [128, NBLK], dt.float32, kind="ExternalInput").ap()
    t_cp = nc.dram_tensor("cp", [128, NBLK * G], dt.bfloat16, kind="ExternalInput").ap()
    t_xti = nc.dram_tensor("xti", [2 * F, NBLK * BLK], dt.float32, kind="ExternalInput").ap()
    t_w1 = nc.dram_tensor("w1", [2 * F, H], dt.float32, kind="ExternalInput").ap()
    t_w2 = nc.dram_tensor("w2", [H, H], dt.float32, kind="ExternalInput").ap()
    t_w3 = nc.dram_tensor("w3", [H, H], dt.float32, kind="ExternalInput").ap()
    t_wl = nc.dram_tensor("wl", [H, C], dt.float32, kind="ExternalInput").ap()
    t_b1 = nc.dram_tensor("b1b", [128, H], dt.float32, kind="ExternalInput").ap()
    t_b2 = nc.dram_tensor("b2b", [128, H], dt.float32, kind="ExternalInput").ap()
    t_id = nc.dram_tensor("ident", [128, 128], dt.float32, kind="ExternalInput").ap()
    t_out = nc.dram_tensor("out", [C, G], dt.float32, kind="ExternalOutput").ap()

    t_min = nc.dram_tensor("mhl_in", [NP, ELEM], dt.bfloat16, kind="Internal").ap()
    t_mfull = nc.dram_tensor("mhl_full", [N, ELEM], dt.bfloat16, kind="Internal",
                             addr_space="Shared").ap()
    debug = os.environ.get("GCN_DEBUG", "0") == "1"
    t_dh1 = t_dh2 = None
    if debug:
        t_dh1 = nc.dram_tensor("dbg_h1", [NP, H], dt.float32, kind="ExternalOutput").ap()
        t_dh2 = nc.dram_tensor("dbg_h2", [NP, H], dt.float32, kind="ExternalOutput").ap()

    qctr = [0]

    def nextq():
        q = qctr[0] % 4
        qctr[0] += 1
        return q

    def group_blocks(g):
        return list(range(g * BPG, min((g + 1) * BPG, NBLK)))

    with tile.TileContext(nc) as tc:
        with tc.tile_pool(name="const", bufs=1) as cpool, \
             tc.tile_pool(name="mown", bufs=1) as mpool, \
             tc.tile_pool(name="h2acc", bufs=1) as apool:
            w1s = cpool.tile([2 * F, H], dt.float32)
            w2s = cpool.tile([H, H], dt.float32)
            w3s = cpool.tile([H, H], dt.float32)
            wls = cpool.tile([H, C], dt.float32)
            b1s = cpool.tile([128, H], dt.float32)
            b2s = cpool.tile([128, H], dt.float32)
            dsos = cpool.tile([128, NBLK], dt.float32)
            cps = cpool.tile([128, NBLK * G], dt.bfloat16)
            ids = cpool.tile([128, 128], dt.float32)
            xts = cpool.tile([2 * F, NBLK * BLK], dt.float32)
            for dst_t, src_t in [(w1s, t_w1), (w2s, t_w2), (w3s, t_w3),
                                 (wls, t_wl), (b1s, t_b1), (b2s, t_b2),
                                 (dsos, t_dso), (cps, t_cp), (ids, t_id),
                                 (xts, t_xti)]:
                nc.sync.dma_start(dst_t[:], src_t[:])

            mown = mpool.tile([128, NBLK * H], dt.bfloat16)
            h2acc = apool.tile([128, NBLK * H], dt.bfloat16)

            # ---------------- phase L1 + quarter collectives ----------------
            with tc.tile_pool(name="g1", bufs=5) as g1p, \
                 tc.tile_pool(name="seg1", bufs=5) as seg1p, \
                 tc.tile_pool(name="idx1", bufs=5) as idx1p, \
                 tc.tile_pool(name="blk1", bufs=4) as blkp, \
                 tc.tile_pool(name="ps_z", bufs=2, space="PSUM") as psz, \
                 tc.tile_pool(name="ps_a", bufs=2, space="PSUM") as psa, \
                 tc.tile_pool(name="ps_a2", bufs=2, space="PSUM") as psa2, \
                 tc.tile_pool(name="ps_b", bufs=2, space="PSUM") as psb:
                for g in range(NGRP):
                    blocks = group_blocks(g)
                    i0 = int(IBASE[g, 0])
                    ilen = int(sum(NIDX[g, c] for c in range(NQ)))
                    p0 = int(PBASE[g, 0])
                    plen = int(sum(POFF[blocks[-1], c] + (T1[blocks[-1], c] - T0[blocks[-1], c])
                                   for c in range(NQ)))
                    it = idx1p.tile([128, (ilen) // 16], dt.int16, tag="idx",
                                    name=f"i1_{g}")
                    st = seg1p.tile([128, plen * BLK], dt.float8e4, tag="seg",
                                    name=f"s1_{g}")
                    nc.sync.dma_start(it[:], t_idx[:, i0 // 16:(i0 + ilen) // 16])
                    nc.sync.dma_start(st[:], t_seg[:, p0 * BLK:(p0 + plen) * BLK])
                    gts = []
                    for c in range(NQ):
                        nidx = int(NIDX[g, c])
                        ntile = int(NTILE[g, c])
                        gt = g1p.tile([128, NT_MAX, ELEM], dt.bfloat16, tag="g",
                                      name=f"g1_{g}_{c}")
                        iof = int(IBASE[g, c]) - i0
                        nc.gpsimd.dma_gather(
                            gt[:, 0:ntile, :], t_xhl[CB[c]:CB[c] + CH[c], :],
                            it[:, iof // 16:(iof + nidx) // 16], nidx, nidx, ELEM,
                            single_packet=False, queue_num=nextq())
                        gts.append(gt)
                    zbank = psz.tile([2 * F, 512], dt.float32, tag="z", name=f"z{g}")
                    for kb, b in enumerate(blocks):
                        reg = zbank[:, kb * 128:(kb + 1) * 128]
                        nmm = sum(int(T1[b, c] - T0[b, c]) for c in range(NQ))
                        k = 0
                        for c in range(NQ):
                            pb = int(PBASE[g, c]) + int(POFF[b, c]) - p0
                            for j in range(int(T1[b, c] - T0[b, c])):
                                tcn = int(T0[b, c]) + j
                                nc.tensor.matmul(
                                    reg,
                                    lhsT=gts[c][:, tcn, 0:2 * F],
                                    rhs=st[:, (pb + j) * BLK:(pb + j + 1) * BLK],
                                    start=(k == 0), stop=(k == nmm - 1))
                                k += 1
                        zs = blkp.tile([2 * F, 128], dt.float32, tag="zs", name=f"zs{g}_{kb}")
                        nc.scalar.copy(zs[:], reg)
                        # self-loop term
                        nc.vector.tensor_tensor(zs[:], zs[:],
                                                xts[:, b * 128:(b + 1) * 128],
                                                op=ALU.add)
                        ph = psa.tile([128, H], dt.float32, tag="ph", name=f"ph{g}_{kb}")
                        nc.tensor.matmul(ph[:], lhsT=zs[:], rhs=w1s[:], start=True, stop=True)
                        h1 = blkp.tile([128, H], dt.float32, tag="h1", name=f"h1_{g}_{kb}")
                        nc.scalar.activation(h1[:], ph[:], AF.Copy,
                                             scale=dsos[:, b:b + 1])
                        nc.vector.tensor_tensor(h1[:], h1[:], b1s[:], op=ALU.add)
                        nc.scalar.activation(h1[:], h1[:], AF.Relu)
                        pt = psb.tile([128, 128], dt.float32, tag="pt", name=f"pt{g}_{kb}")
                        nc.tensor.transpose(pt[:], h1[:], ids[:])
                        h1f = blkp.tile([128, 128], dt.float32, tag="h1f", name=f"h1f{g}_{kb}")
                        nc.vector.tensor_copy(h1f[:], pt[:])
                        pm = psa2.tile([128, H], dt.float32, tag="pm", name=f"pm{g}_{kb}")
                        nc.tensor.matmul(pm[:], lhsT=h1f[:], rhs=w2s[:], start=True, stop=True)
                        # m2 = dis * (h1 @ W2) in bf16, kept on-chip + written out
                        nc.scalar.activation(mown[:, b * H:(b + 1) * H], pm[:],
                                             AF.Copy, scale=dsos[:, b:b + 1])
                        rb = min(128, NP - b * 128)
                        nc.sync.dma_start(t_min[b * 128:b * 128 + rb, :],
                                          mown[0:rb, b * H:(b + 1) * H])
                        if debug:
                            nc.sync.dma_start(t_dh1[b * 128:b * 128 + rb, :], h1[0:rb, :])
                    # quarter collective
                    for q in range(NQ):
                        if QGRP[q] == g:
                            nc.gpsimd.collective_compute(
                                "AllGather", mybir.AluOpType.bypass,
                                replica_groups=[list(range(P))],
                                ins=[t_min[QO[q]:QO[q] + QS[q], :]],
                                outs=[t_mfull[CB[q]:CB[q] + CH[q], :]])

            # ---------------- phase L2 (chunk-major) ----------------
            with tc.tile_pool(name="g2", bufs=5) as g2p, \
                 tc.tile_pool(name="seg2", bufs=5) as seg2p, \
                 tc.tile_pool(name="idx2", bufs=5) as idx2p, \
                 tc.tile_pool(name="blk2", bufs=4) as blk2p, \
                 tc.tile_pool(name="pool_acc", bufs=1) as pap, \
                 tc.tile_pool(name="ps_agg", bufs=2, space="PSUM") as psagg, \
                 tc.tile_pool(name="ps_c", bufs=2, space="PSUM") as psc, \
                 tc.tile_pool(name="ps_d", bufs=1, space="PSUM") as psd, \
                 tc.tile_pool(name="ps_e", bufs=1, space="PSUM") as pse:
                pacc = pap.tile([128, G], dt.float32)
                for c in range(NQ):
                    for g in range(NGRP):
                        blocks = group_blocks(g)
                        nidx = int(NIDX[g, c])
                        ntile = int(NTILE[g, c])
                        i0 = int(IBASE[g, c])
                        p0 = int(PBASE[g, c])
                        plen = int(POFF[blocks[-1], c] + (T1[blocks[-1], c] - T0[blocks[-1], c]))
                        it = idx2p.tile([128, NT_MAX * 8], dt.int16, tag="idx",
                                        name=f"i2_{g}_{c}")
                        st = seg2p.tile([128, plen * BLK], dt.float8e4, tag="seg",
                                        name=f"s2_{g}_{c}")
                        nc.sync.dma_start(it[:, 0:nidx // 16],
                                          t_idx[:, i0 // 16:(i0 + nidx) // 16])
                        nc.sync.dma_start(st[:], t_seg[:, p0 * BLK:(p0 + plen) * BLK])
                        gt = g2p.tile([128, NT_MAX, ELEM], dt.bfloat16, tag="g",
                                      name=f"g2_{g}_{c}")
                        nc.gpsimd.dma_gather(
                            gt[:, 0:ntile, :], t_mfull[CB[c]:CB[c] + CH[c], :],
                            it[:, 0:nidx // 16], nidx, nidx, ELEM,
                            single_packet=False, queue_num=nextq())
                        bank = psagg.tile([128, 512], dt.float32, tag="agg",
                                          name=f"ab{g}_{c}")
                        for kb, b in enumerate(blocks):
                            reg = bank[:, kb * 128:(kb + 1) * 128]
                            nmm = int(T1[b, c] - T0[b, c])
                            pb = int(PBASE[g, c]) + int(POFF[b, c]) - p0
                            for j in range(nmm):
                                tcn = int(T0[b, c]) + j
                                nc.tensor.matmul(
                                    reg,
                                    lhsT=st[:, (pb + j) * BLK:(pb + j + 1) * BLK],
                                    rhs=gt[:, tcn, :],
                                    start=(j == 0), stop=(j == nmm - 1))
                            acc = h2acc[:, b * H:(b + 1) * H]
                            if c == 0:
                                nc.scalar.copy(acc, reg)
                            else:
                                nc.vector.tensor_tensor(acc, reg, acc, op=ALU.add)
                            if c == NQ - 1:
                                # epilogue: + self m2, scale, bias, relu
                                h2 = blk2p.tile([128, H], dt.float32, tag="h2",
                                                name=f"h2_{g}_{kb}")
                                nc.vector.tensor_tensor(
                                    h2[:], acc, mown[:, b * H:(b + 1) * H], op=ALU.add)
                                nc.scalar.activation(h2[:], h2[:], AF.Copy,
                                                     scale=dsos[:, b:b + 1])
                                nc.vector.tensor_tensor(h2[:], h2[:], b2s[:], op=ALU.add)
                                nc.scalar.activation(h2[:], h2[:], AF.Relu)
                                if debug:
                                    rb = min(128, NP - b * 128)
                                    nc.sync.dma_start(t_dh2[b * 128:b * 128 + rb, :],
                                                      h2[0:rb, :])
                                pt2 = psc.tile([128, 128], dt.float32, tag="pt2",
                                               name=f"pt2{g}_{kb}")
                                nc.tensor.transpose(pt2[:], h2[:], ids[:])
                                h2f = blk2p.tile([128, 128], dt.float32, tag="h2f",
                                                 name=f"h2f{g}_{kb}")
                                nc.vector.tensor_copy(h2f[:], pt2[:])
                                pm3 = psd.tile([128, H], dt.float32, tag="pm3",
                                               name=f"pm3{g}_{kb}")
                                nc.tensor.matmul(pm3[:], lhsT=h2f[:], rhs=w3s[:],
                                                 start=True, stop=True)
                                m3 = blk2p.tile([128, H], dt.bfloat16, tag="m3",
                                                name=f"m3_{g}_{kb}")
                                nc.scalar.copy(m3[:], pm3[:])
                                pp = pse.tile([128, G], dt.float32, tag="pp",
                                              name=f"pp{g}_{kb}")
                                nc.tensor.matmul(pp[:], lhsT=m3[:],
                                                 rhs=cps[:, b * G:(b + 1) * G],
                                                 start=True, stop=True)
                                if g == 0 and kb == 0:
                                    nc.scalar.copy(pacc[:], pp[:])
                                else:
                                    nc.vector.tensor_tensor(pacc[:], pp[:], pacc[:],
                                                            op=ALU.add)

                # tail: per-core head partial (summed on host)
                with tc.tile_pool(name="tail", bufs=1) as tp, \
                     tc.tile_pool(name="ps_o", bufs=1, space="PSUM") as pso:
                    po = pso.tile([C, G], dt.float32)
                    nc.tensor.matmul(po[:], lhsT=wls[:], rhs=pacc[:], start=True, stop=True)
                    osb = tp.tile([C, G], dt.float32)
                    nc.scalar.copy(osb[:], po[:])
                    nc.sync.dma_start(t_out[:], osb[:])

    nc.compile()
    return nc


def kernel(**inputs):
    from concourse.bass_utils import run_bass_kernel_spmd

    x = np.asarray(inputs["x"], np.float32)
    edge_index = np.asarray(inputs["edge_index"], np.int64)
    batch = np.asarray(inputs["batch"], np.int64)
    W1 = np.asarray(inputs["W1"], np.float32)
    b1 = np.asarray(inputs["b1"], np.float32)
    W2 = np.asarray(inputs["W2"], np.float32)
    b2 = np.asarray(inputs["b2"], np.float32)
    W3 = np.asarray(inputs["W3"], np.float32)
    b3 = np.asarray(inputs["b3"], np.float32)
    Wlin = np.asarray(inputs["Wlin"], np.float32)
    blin = np.asarray(inputs["blin"], np.float32)

    meta, xhl, per_core, _dis = _host_prep(x, edge_index, batch)

    key = "nc"
    if key not in _CACHE:
        _CACHE[key] = _build(meta)
    nc = _CACHE[key]

    in_maps = []
    for i in range(P):
        pc = per_core[i]
        in_maps.append({
            "xhl": xhl,
            "idxw": pc["idxw"], "seg": pc["seg"], "dso": pc["dso"],
            "cp": pc["cp"], "xti": pc["xti"],
            "w1": np.vstack([W1, W1]).astype(np.float32), "w2": W2, "w3": W3,
            "wl": Wlin,
            "b1b": np.tile(b1, (128, 1)).astype(np.float32),
            "b2b": np.tile(b2, (128, 1)).astype(np.float32),
            "ident": np.eye(128, dtype=np.float32),
        })

    trace = os.environ.get("GCN_TRACE", "0") == "1"
    res = run_bass_kernel_spmd(nc, in_maps, core_ids=list(range(P)), trace=trace)
    if trace:
        print("HW exec time:", res.exec_time_ns, "ns")
        if res.instructions_and_trace:
            print("trace:", res.instructions_and_trace[1])
    parts = [np.asarray(res.results[i]["out"], np.float32) for i in range(P)]
    out = np.sum(parts, axis=0)  # [C, G]
    out += (Wlin.T @ b3)[:, None] + blin[:, None]
    return np.ascontiguousarray(out.T).astype(np.float32)


# revision 7
# speedup vs baseline: 1.8345x; 1.0463x over previous
"""GCN (3-layer + mean-pool + linear head) on 8 TRN2 NeuronCores.

v2 strategy (dst-sharded message passing, packed gathers, pipelined collective):
  - Nodes split into 8 slices of 12500; core i owns dst slice i (98 blocks of
    128) and the edges pointing into it. Self-loops are NOT gathered: their
    contribution is added algebraically from data already on-chip.
  - Table row order R(src) = (core, quarter, offset) so the AllGather of m2
    can be split into 4 quarter collectives that fire during the L1 block
    loop; L2 runs chunk-major with SBUF bf16 accumulators so chunk-q work
    only depends on collective q.
  - L1 aggregates x-tilde = dis*x (hi/lo bf16 in 256B rows, host-precomputed
    table) first, then applies W1: (S x) W1 == S (x W1).
  - Messages m2 = dis*(h1 W2) stored as single bf16 (256B rows).
  - Gather streams are PACKED per (group, chunk): blocks share boundary tile
    columns; the one-hot seg pieces route edges to the right dst block, so
    descriptors ~= edges (no per-block ceil-to-128 padding).
  - Layer 3 + mean-pool collapse into a dense matmul with host-built C'
    (index/degree data only). Final head partials [C, G] are summed on host
    (removes the tail AllReduce); bias constants folded on host.
"""
import os
import sys

sys.path.insert(0, "/opt/trn_rl_repo")

import numpy as np
import ml_dtypes

N = 100000
E = 1600000
F = 3
H = 128
C = 4
G = 64
P = 8
NP = N // P                 # 12500
BLK = 128
NBLK = (NP + BLK - 1) // BLK    # 98
BPG = 4
NGRP = (NBLK + BPG - 1) // BPG  # 25
NQ = 4
QBLKS = [25, 25, 25, 23]        # blocks per quarter
QS = [3200, 3200, 3200, 2900]   # rows per quarter per core
QO = [0, 3200, 6400, 9600]
CH = [8 * s for s in QS]        # chunk table sizes
CB = [0, 25600, 51200, 76800]   # chunk table bases
QGRP = [6, 12, 18, 24]          # collective q fires after this group
ELEM = 128                      # bf16 elems per table row (256B)

_CACHE = {}


def _bf16_hilo(a):
    hi = a.astype(ml_dtypes.bfloat16)
    lo = (a - hi.astype(np.float32)).astype(ml_dtypes.bfloat16)
    return hi, lo


def _host_prep(x, edge_index, batch):
    f8 = ml_dtypes.float8_e4m3
    src = np.asarray(edge_index[0], np.int64)
    dst = np.asarray(edge_index[1], np.int64)
    deg = (np.bincount(dst, minlength=N) + 1).astype(np.float32)  # + self-loop
    dis = deg ** np.float32(-0.5)

    # table row mapping (core, quarter, offset)
    def rowmap(s):
        i = s // NP
        r = s % NP
        q = np.minimum(r // 3200, 3)
        qs = np.asarray(QS, np.int64)[q]
        qo = np.asarray(QO, np.int64)[q]
        cb = np.asarray(CB, np.int64)[q]
        srel = i * qs + (r - qo)
        return q, srel, cb + srel

    # x-tilde table [N, ELEM] bf16: cols 0:3 hi, 3:6 lo, rest zero
    xt = dis[:, None] * np.asarray(x, np.float32)
    _, _, trow = rowmap(np.arange(N, dtype=np.int64))
    xhl = np.zeros((N, ELEM), ml_dtypes.bfloat16)
    hi, lo = _bf16_hilo(xt)
    xhl[trow, 0:F] = hi
    xhl[trow, F:2 * F] = lo

    # C' pooled matrix (uses FULL edge set incl self-loops)
    batch = np.asarray(batch, np.int64)
    cntg = np.bincount(batch, minlength=G).astype(np.float32)
    srcL = np.concatenate([src, np.arange(N, dtype=np.int64)])
    dstL = np.concatenate([dst, np.arange(N, dtype=np.int64)])
    cmat = np.zeros((G, N), np.float32)
    np.add.at(cmat, (batch[dstL], srcL), dis[srcL] * dis[dstL])
    cmat /= np.maximum(cntg, 1.0)[:, None]

    # per-edge core/block/chunk decomposition (NO self-loops)
    core = dst // NP
    blk = (dst % NP) // BLK
    dloc = (dst % NP) % BLK
    ec, es, _ = rowmap(src)

    gi = blk // BPG
    bi = blk % BPG
    key = ((core * NGRP + gi) * NQ + ec) * BPG + bi
    order = np.argsort(key, kind="stable")

    # per-core per-(g,c,b) counts
    kk = ((core * NBLK + blk) * NQ + ec)
    cnt = np.bincount(kk, minlength=P * NBLK * NQ).reshape(P, NBLK, NQ)

    # per (g,c): packed stream; per-core block starts
    start = np.zeros((P, NBLK, NQ), np.int64)
    for g in range(NGRP):
        bs = range(g * BPG, min((g + 1) * BPG, NBLK))
        run = np.zeros((P, NQ), np.int64)
        for b in bs:
            start[:, b, :] = run
            run += cnt[:, b, :]
    cnt_pad = np.zeros((P, NGRP * BPG, NQ), np.int64)
    cnt_pad[:, :NBLK, :] = cnt
    cnt_gc = cnt_pad.reshape(P, NGRP, BPG, NQ).sum(axis=2)   # [P, g, c]
    NIDX = (-(-cnt_gc.max(axis=0) // BLK) * BLK).astype(np.int64)  # [g, c]
    NIDX = np.maximum(NIDX, BLK)
    NTILE = NIDX // BLK

    # piece (block-column) program, shared across cores
    T0 = (start.min(axis=0) // BLK).astype(np.int64)              # [NBLK, NQ]
    T1 = (-(-(start + cnt).max(axis=0) // BLK)).astype(np.int64)  # [NBLK, NQ]
    T1 = np.maximum(T1, T0 + 1)
    SPAN = T1 - T0

    IBASE = np.zeros((NGRP, NQ), np.int64)   # idx stream base (in idx units)
    PBASE = np.zeros((NGRP, NQ), np.int64)   # seg piece base
    POFF = np.zeros((NBLK, NQ), np.int64)    # piece offset within (g,c)
    it_, pt_ = 0, 0
    for g in range(NGRP):
        for c in range(NQ):
            IBASE[g, c] = it_
            PBASE[g, c] = pt_
            po = 0
            for b in range(g * BPG, min((g + 1) * BPG, NBLK)):
                POFF[b, c] = po
                po += int(SPAN[b, c])
            it_ += int(NIDX[g, c])
            pt_ += po
    ITOT, PTOT = it_, pt_

    meta = dict(NIDX=NIDX, NTILE=NTILE, T0=T0, T1=T1, IBASE=IBASE,
                PBASE=PBASE, POFF=POFF, ITOT=ITOT, PTOT=PTOT,
                NT_MAX=int(NTILE.max()))

    # per-core streams
    per_core = []
    cbound = np.searchsorted(core[order], np.arange(P + 1))
    for i in range(P):
        sel = order[cbound[i]:cbound[i + 1]]
        eb, ecc, edd, ess = blk[sel], ec[sel], dloc[sel], es[sel]
        bckt = (eb * NQ + ecc)
        # rank within (b, c) bucket: edges already sorted by (g,c,b) so each
        # (b,c) bucket is contiguous; rank = position - bucket start
        bc_cnt = np.bincount(bckt, minlength=NBLK * NQ)
        bc_start = np.zeros(NBLK * NQ, np.int64)
        ord2 = np.argsort(bckt, kind="stable")
        inv = np.empty_like(ord2)
        inv[ord2] = np.arange(len(sel))
        bc_start[1:] = np.cumsum(bc_cnt)[:-1]
        rank_sorted = np.arange(len(sel)) - bc_start[bckt[ord2]]
        rank = np.empty(len(sel), np.int64)
        rank[ord2] = rank_sorted
        # position within the packed (g,c) stream
        pos = start[i][eb, ecc] + rank
        gidx = eb // BPG
        spos = IBASE[gidx, ecc] + pos           # global idx-stream position
        idx16 = np.zeros(ITOT, np.int16)
        idx16[spos] = ess.astype(np.int16)
        # wrap for gpsimd: [128, ITOT//16], replicated 8x over partition rows
        wr = idx16.reshape(-1, 16).T            # [16, ITOT//16]
        idxw = np.tile(wr, (8, 1)).astype(np.int16)
        # seg pieces
        tc = pos // BLK
        prow = pos % BLK
        pidx = PBASE[gidx, ecc] + POFF[eb, ecc] + (tc - T0[eb, ecc])
        seg = np.zeros((BLK, PTOT * BLK), f8)
        seg[prow, pidx * BLK + edd] = np.float32(1.0).astype(f8)
        # dis per block layout [128, NBLK]
        own = dis[i * NP:(i + 1) * NP]
        dso = np.zeros(NBLK * BLK, np.float32)
        dso[:NP] = own
        dso = dso.reshape(NBLK, BLK).T.copy()
        # cp [128, NBLK*G] bf16
        cpc = np.zeros((NBLK * BLK, G), np.float32)
        cpc[:NP, :] = cmat[:, i * NP:(i + 1) * NP].T
        cp = cpc.reshape(NBLK, BLK, G).transpose(1, 0, 2).reshape(BLK, NBLK * G)
        cp = cp.astype(ml_dtypes.bfloat16)
        # x-tilde^T hi/lo for the L1 self term [6, NBLK*128] f32
        xti = np.zeros((2 * F, NBLK * BLK), np.float32)
        hi_i, lo_i = _bf16_hilo(xt[i * NP:(i + 1) * NP])
        xti[0:F, :NP] = hi_i.astype(np.float32).T
        xti[F:2 * F, :NP] = lo_i.astype(np.float32).T
        per_core.append(dict(idxw=idxw, seg=np.asarray(seg), dso=dso, cp=cp,
                             xti=xti))

    return meta, xhl, per_core, dis


def _build(meta):
    import concourse.bacc as bacc
    import concourse.mybir as mybir
    import concourse.tile as tile

    dt = mybir.dt
    AF = mybir.ActivationFunctionType
    ALU = mybir.AluOpType

    NIDX = meta["NIDX"]
    NTILE = meta["NTILE"]
    T0, T1 = meta["T0"], meta["T1"]
    IBASE, PBASE, POFF = meta["IBASE"], meta["PBASE"], meta["POFF"]
    ITOT, PTOT = meta["ITOT"], meta["PTOT"]
    NT_MAX = meta["NT_MAX"]

    nc = bacc.Bacc("TRN2", target_bir_lowering=False, debug=False,
                   num_devices=P, num_swdge_queues=4)

    t_xhl = nc.dram_tensor("xhl", [N, ELEM], dt.bfloat16, kind="ExternalInput").ap()
    t_idx = nc.dram_tensor("idxw", [128, ITOT // 16], dt.int16, kind="ExternalInput").ap()
    t_seg = nc.dram_tensor("seg", [128, PTOT * BLK], dt.float8e4, kind="ExternalInput").ap()
    t_dso = nc.dram_tensor("dso", [128, NBLK], dt.float32, kind="ExternalInput").ap()
    t_cp = nc.dram_tensor("cp", [128, NBLK * G], dt.bfloat16, kind="ExternalInput").ap()
    t_xti = nc.dram_tensor("xti", [2 * F, NBLK * BLK], dt.float32, kind="ExternalInput").ap()
    t_w1 = nc.dram_tensor("w1", [2 * F, H], dt.float32, kind="ExternalInput").ap()
    t_w2 = nc.dram_tensor("w2", [H, H], dt.float32, kind="ExternalInput").ap()
    t_w3 = nc.dram_tensor("w3", [H, H], dt.float32, kind="ExternalInput").ap()
    t_wl = nc.dram_tensor("wl", [H, C], dt.float32, kind="ExternalInput").ap()
    t_b1 = nc.dram_tensor("b1b", [128, H], dt.float32, kind="ExternalInput").ap()
    t_b2 = nc.dram_tensor("b2b", [128, H], dt.float32, kind="ExternalInput").ap()
    t_id = nc.dram_tensor("ident", [128, 128], dt.float32, kind="ExternalInput").ap()
    t_out = nc.dram_tensor("out", [C, G], dt.float32, kind="ExternalOutput").ap()

    t_min = nc.dram_tensor("mhl_in", [NP, ELEM], dt.bfloat16, kind="Internal").ap()
    t_mfull = nc.dram_tensor("mhl_full", [N, ELEM], dt.bfloat16, kind="Internal",
                             addr_space="Shared").ap()
    debug = os.environ.get("GCN_DEBUG", "0") == "1"
    t_dh1 = t_dh2 = None
    if debug:
        t_dh1 = nc.dram_tensor("dbg_h1", [NP, H], dt.float32, kind="ExternalOutput").ap()
        t_dh2 = nc.dram_tensor("dbg_h2", [NP, H], dt.float32, kind="ExternalOutput").ap()

    qctr = [0]

    def nextq():
        q = qctr[0] % 4
        qctr[0] += 1
        return q

    def group_blocks(g):
        return list(range(g * BPG, min((g + 1) * BPG, NBLK)))

    # L2 interleave gating: chunk-c L2 items may be emitted into the Pool
    # program only after L1 group GATE[c] (collective c must have landed).
    GATE = [10, 16, 22, NGRP]

    with tile.TileContext(nc) as tc:
        with tc.tile_pool(name="const", bufs=1) as cpool, \
             tc.tile_pool(name="mown", bufs=1) as mpool, \
             tc.tile_pool(name="h2acc", bufs=1) as apool, \
             tc.tile_pool(name="g1", bufs=5) as g1p, \
             tc.tile_pool(name="seg1", bufs=5) as seg1p, \
             tc.tile_pool(name="blk1", bufs=4) as blkp, \
             tc.tile_pool(name="g2", bufs=5) as g2p, \
             tc.tile_pool(name="seg2", bufs=5) as seg2p, \
             tc.tile_pool(name="blk2", bufs=4) as blk2p, \
             tc.tile_pool(name="pool_acc", bufs=1) as pap, \
             tc.tile_pool(name="ps_w", bufs=2, space="PSUM") as psw, \
             tc.tile_pool(name="ps_m", bufs=3, space="PSUM") as psm, \
             tc.tile_pool(name="ps_t", bufs=2, space="PSUM") as pst, \
             tc.tile_pool(name="ps_e", bufs=1, space="PSUM") as pse:
            w1s = cpool.tile([2 * F, H], dt.float32)
            w2s = cpool.tile([H, H], dt.float32)
            w3s = cpool.tile([H, H], dt.float32)
            wls = cpool.tile([H, C], dt.float32)
            b1s = cpool.tile([128, H], dt.float32)
            b2s = cpool.tile([128, H], dt.float32)
            dsos = cpool.tile([128, NBLK], dt.float32)
            cps = cpool.tile([128, NBLK * G], dt.bfloat16)
            ids = cpool.tile([128, 128], dt.float32)
            idxs = cpool.tile([128, ITOT // 16], dt.int16)
            for dst_t, src_t in [(w1s, t_w1), (w2s, t_w2), (w3s, t_w3),
                                 (wls, t_wl), (b1s, t_b1), (b2s, t_b2),
                                 (dsos, t_dso), (cps, t_cp), (ids, t_id),
                                 (idxs, t_idx)]:
                nc.sync.dma_start(dst_t[:], src_t[:])

            mown = mpool.tile([128, NBLK * H], dt.bfloat16)
            h2acc = apool.tile([128, NBLK * H], dt.bfloat16)
            pacc = pap.tile([128, G], dt.float32)
            first_pp = [True]

            def emit_l1_group(g):
                blocks = group_blocks(g)
                gts = []
                sts = []
                p0s = []
                for c in range(NQ):
                    nidx = int(NIDX[g, c])
                    ntile = int(NTILE[g, c])
                    p0c = int(PBASE[g, c])
                    plen = int(POFF[blocks[-1], c] + (T1[blocks[-1], c] - T0[blocks[-1], c]))
                    st = seg1p.tile([128, plen * BLK], dt.float8e4, tag="seg",
                                    name=f"s1_{g}_{c}")
                    nc.sync.dma_start(st[:], t_seg[:, p0c * BLK:(p0c + plen) * BLK])
                    gt = g1p.tile([128, NT_MAX, ELEM], dt.bfloat16, tag="g",
                                  name=f"g1_{g}_{c}")
                    i0 = int(IBASE[g, c])
                    nc.gpsimd.dma_gather(
                        gt[:, 0:ntile, :], t_xhl[CB[c]:CB[c] + CH[c], :],
                        idxs[:, i0 // 16:(i0 + nidx) // 16], nidx, nidx, ELEM,
                        single_packet=False, queue_num=nextq())
                    gts.append(gt)
                    sts.append(st)
                    p0s.append(p0c)
                zbank = psw.tile([128, 512], dt.float32, tag="w", name=f"z{g}")
                for kb, b in enumerate(blocks):
                    reg = zbank[0:2 * F, kb * 128:(kb + 1) * 128]
                    nmm = sum(int(T1[b, c] - T0[b, c]) for c in range(NQ))
                    k = 0
                    for c in range(NQ):
                        pb = int(POFF[b, c])
                        for j in range(int(T1[b, c] - T0[b, c])):
                            tcn = int(T0[b, c]) + j
                            nc.tensor.matmul(
                                reg,
                                lhsT=gts[c][:, tcn, 0:2 * F],
                                rhs=sts[c][:, (pb + j) * BLK:(pb + j + 1) * BLK],
                                start=(k == 0), stop=(k == nmm - 1))
                            k += 1
                    zs = blkp.tile([2 * F, 128], dt.float32, tag="zs", name=f"zs{g}_{kb}")
                    nc.scalar.copy(zs[:], reg)
                    xtb = blkp.tile([2 * F, 128], dt.float32, tag="xtb", name=f"xtb{g}_{kb}")
                    nc.sync.dma_start(xtb[:], t_xti[:, b * 128:(b + 1) * 128])
                    ph = psm.tile([128, H], dt.float32, tag="m", name=f"ph{g}_{kb}")
                    nc.tensor.matmul(ph[:], lhsT=zs[:], rhs=w1s[:], start=True, stop=False)
                    nc.tensor.matmul(ph[:], lhsT=xtb[:], rhs=w1s[:], start=False, stop=True)
                    h1 = blkp.tile([128, H], dt.float32, tag="h1", name=f"h1_{g}_{kb}")
                    nc.scalar.activation(h1[:], ph[:], AF.Copy,
                                         scale=dsos[:, b:b + 1])
                    nc.vector.tensor_tensor(h1[:], h1[:], b1s[:], op=ALU.add)
                    nc.scalar.activation(h1[:], h1[:], AF.Relu)
                    pt = pst.tile([128, 128], dt.float32, tag="t", name=f"pt{g}_{kb}")
                    nc.tensor.transpose(pt[:], h1[:], ids[:])
                    h1f = blkp.tile([128, 128], dt.float32, tag="h1f", name=f"h1f{g}_{kb}")
                    nc.vector.tensor_copy(h1f[:], pt[:])
                    pm = psm.tile([128, H], dt.float32, tag="m", name=f"pm{g}_{kb}")
                    nc.tensor.matmul(pm[:], lhsT=h1f[:], rhs=w2s[:], start=True, stop=True)
                    nc.scalar.activation(mown[:, b * H:(b + 1) * H], pm[:],
                                         AF.Copy, scale=dsos[:, b:b + 1])
                    rb = min(128, NP - b * 128)
                    nc.sync.dma_start(t_min[b * 128:b * 128 + rb, :],
                                      mown[0:rb, b * H:(b + 1) * H])
                    if debug:
                        nc.sync.dma_start(t_dh1[b * 128:b * 128 + rb, :], h1[0:rb, :])
                for q in range(NQ):
                    if QGRP[q] == g:
                        nc.gpsimd.collective_compute(
                            "AllGather", mybir.AluOpType.bypass,
                            replica_groups=[list(range(P))],
                            ins=[t_min[QO[q]:QO[q] + QS[q], :]],
                            outs=[t_mfull[CB[q]:CB[q] + CH[q], :]])

            def emit_l2_item(c, g):
                blocks = group_blocks(g)
                nidx = int(NIDX[g, c])
                ntile = int(NTILE[g, c])
                i0 = int(IBASE[g, c])
                p0 = int(PBASE[g, c])
                plen = int(POFF[blocks[-1], c] + (T1[blocks[-1], c] - T0[blocks[-1], c]))
                st = seg2p.tile([128, plen * BLK], dt.float8e4, tag="seg",
                                name=f"s2_{g}_{c}")
                nc.sync.dma_start(st[:], t_seg[:, p0 * BLK:(p0 + plen) * BLK])
                gt = g2p.tile([128, NT_MAX, ELEM], dt.bfloat16, tag="g",
                              name=f"g2_{g}_{c}")
                nc.gpsimd.dma_gather(
                    gt[:, 0:ntile, :], t_mfull[CB[c]:CB[c] + CH[c], :],
                    idxs[:, i0 // 16:(i0 + nidx) // 16], nidx, nidx, ELEM,
                    single_packet=False, queue_num=nextq())
                bank = psw.tile([128, 512], dt.float32, tag="w",
                                name=f"ab{g}_{c}")
                for kb, b in enumerate(blocks):
                    reg = bank[:, kb * 128:(kb + 1) * 128]
                    nmm = int(T1[b, c] - T0[b, c])
                    pb = int(PBASE[g, c]) + int(POFF[b, c]) - p0
                    for j in range(nmm):
                        tcn = int(T0[b, c]) + j
                        nc.tensor.matmul(
                            reg,
                            lhsT=st[:, (pb + j) * BLK:(pb + j + 1) * BLK],
                            rhs=gt[:, tcn, :],
                            start=(j == 0), stop=(j == nmm - 1))
                    acc = h2acc[:, b * H:(b + 1) * H]
                    if c == 0:
                        nc.scalar.copy(acc, reg)
                    else:
                        nc.vector.tensor_tensor(acc, reg, acc, op=ALU.add)
                    if c == NQ - 1:
                        h2 = blk2p.tile([128, H], dt.float32, tag="h2",
                                        name=f"h2_{g}_{kb}")
                        nc.vector.tensor_tensor(
                            h2[:], acc, mown[:, b * H:(b + 1) * H], op=ALU.add)
                        nc.scalar.activation(h2[:], h2[:], AF.Copy,
                                             scale=dsos[:, b:b + 1])
                        nc.vector.tensor_tensor(h2[:], h2[:], b2s[:], op=ALU.add)
                        nc.scalar.activation(h2[:], h2[:], AF.Relu)
                        if debug:
                            rb = min(128, NP - b * 128)
                            nc.sync.dma_start(t_dh2[b * 128:b * 128 + rb, :],
                                              h2[0:rb, :])
                        pt2 = pst.tile([128, 128], dt.float32, tag="t",
                                       name=f"pt2{g}_{kb}")
                        nc.tensor.transpose(pt2[:], h2[:], ids[:])
                        h2f = blk2p.tile([128, 128], dt.float32, tag="h2f",
                                         name=f"h2f{g}_{kb}")
                        nc.vector.tensor_copy(h2f[:], pt2[:])
                        pm3 = psm.tile([128, H], dt.float32, tag="m",
                                       name=f"pm3{g}_{kb}")
                        nc.tensor.matmul(pm3[:], lhsT=h2f[:], rhs=w3s[:],
                                         start=True, stop=True)
                        m3 = blk2p.tile([128, H], dt.bfloat16, tag="m3",
                                        name=f"m3_{g}_{kb}")
                        nc.scalar.copy(m3[:], pm3[:])
                        pp = pse.tile([128, G], dt.float32, tag="e",
                                      name=f"pp{g}_{kb}")
                        nc.tensor.matmul(pp[:], lhsT=m3[:],
                                         rhs=cps[:, b * G:(b + 1) * G],
                                         start=True, stop=True)
                        if first_pp[0]:
                            nc.scalar.copy(pacc[:], pp[:])
                            first_pp[0] = False
                        else:
                            nc.vector.tensor_tensor(pacc[:], pp[:], pacc[:],
                                                    op=ALU.add)

            # merged emission: L1 groups with L2 items woven in
            weave = os.environ.get("GCN_WEAVE", "0") == "1"
            l2q = [(c, g) for c in range(NQ) for g in range(NGRP)]
            qi = 0
            for g in range(NGRP):
                emit_l1_group(g)
                while weave and qi < len(l2q) and l2q[qi][0] < NQ - 1 \
                        and g >= GATE[l2q[qi][0]] and qi < 2 * (g - 9):
                    emit_l2_item(*l2q[qi])
                    qi += 1
            while qi < len(l2q):
                emit_l2_item(*l2q[qi])
                qi += 1

            # tail: per-core head partial (summed on host)
            with tc.tile_pool(name="tail", bufs=1) as tp:
                po = pse.tile([C, G], dt.float32, tag="e", name="po")
                nc.tensor.matmul(po[:], lhsT=wls[:], rhs=pacc[:], start=True, stop=True)
                osb = tp.tile([C, G], dt.float32)
                nc.scalar.copy(osb[:], po[:])
                nc.sync.dma_start(t_out[:], osb[:])

    nc.compile()
    return nc


def kernel(**inputs):
    from concourse.bass_utils import run_bass_kernel_spmd

    x = np.asarray(inputs["x"], np.float32)
    edge_index = np.asarray(inputs["edge_index"], np.int64)
    batch = np.asarray(inputs["batch"], np.int64)
    W1 = np.asarray(inputs["W1"], np.float32)
    b1 = np.asarray(inputs["b1"], np.float32)
    W2 = np.asarray(inputs["W2"], np.float32)
    b2 = np.asarray(inputs["b2"], np.float32)
    W3 = np.asarray(inputs["W3"], np.float32)
    b3 = np.asarray(inputs["b3"], np.float32)
    Wlin = np.asarray(inputs["Wlin"], np.float32)
    blin = np.asarray(inputs["blin"], np.float32)

    meta, xhl, per_core, _dis = _host_prep(x, edge_index, batch)

    key = "nc"
    if key not in _CACHE:
        _CACHE[key] = _build(meta)
    nc = _CACHE[key]

    in_maps = []
    for i in range(P):
        pc = per_core[i]
        in_maps.append({
            "xhl": xhl,
            "idxw": pc["idxw"], "seg": pc["seg"], "dso": pc["dso"],
            "cp": pc["cp"], "xti": pc["xti"],
            "w1": np.vstack([W1, W1]).astype(np.float32), "w2": W2, "w3": W3,
            "wl": Wlin,
            "b1b": np.tile(b1, (128, 1)).astype(np.float32),
            "b2b": np.tile(b2, (128, 1)).astype(np.float32),
            "ident": np.eye(128, dtype=np.float32),
        })

    trace = os.environ.get("GCN_TRACE", "0") == "1"
    res = run_bass_kernel_spmd(nc, in_maps, core_ids=list(range(P)), trace=trace)
    if trace:
        print("HW exec time:", res.exec_time_ns, "ns")
        if res.instructions_and_trace:
            print("trace:", res.instructions_and_trace[1])
    parts = [np.asarray(res.results[i]["out"], np.float32) for i in range(P)]
    out = np.sum(parts, axis=0)  # [C, G]
    out += (Wlin.T @ b3)[:, None] + blin[:, None]
    return np.ascontiguousarray(out.T).astype(np.float32)
